# revision 1
# baseline (speedup 1.0000x reference)
"""Trainium2 Bass kernel for nn_BidirectionalTrustModel (histogram_binning).

Computes, per observation sequence n (N = 500000, T = 20, BINS = 12):
  1. capability edge c[n]: sequential fold over t of
       c = max(c, d)  if perf==[0,1]
       c = min(c, d)  if perf[...,0]==1
       c            otherwise
  2. trust[n] = sum_k t_k * m_k / sum_k m_k  over 12 bin centers s_k,
       m_k = (c <= s_k),  t_k = (1 + exp(beta*(dpred - s_k)))**(-zeta^2)

Only inptasksperf, difficulties_obs, difficulties_pred, betas, zetas are used
(the other inputs are dead in the reference computation).

Device mapping (pure data parallel over 8 cores, no collectives):
  - per-core slice of 62500 sequences, padded to 62720 = 128 partitions x 490
  - phase A: lo/hi penalty encoding with EXACT trigger values via
      u  = (p1 - 1) + d          (trigger: +0.0 keeps d bit-exact)
      lo = (p0 * -1) + u
      hi = (p0 < 1) + d
    then ONE tensor_tensor_scan(max, min) over a t-inner [128, nseq*21]
    layout (20 steps + 1 reset slot per sequence) computes every c.
  - phase B: per bin exp/ln/exp ACT chain (all funcs in one act table),
    exact is_le masks on DVE, strided tensor_reduce over bins, reciprocal.
"""
import sys

if "/opt/trn_rl_repo" not in sys.path:
    sys.path.insert(0, "/opt/trn_rl_repo")

from contextlib import ExitStack

import numpy as np

import concourse.bacc as bacc
import concourse.bass as bass
import concourse.mybir as mybir
import concourse.tile as tile
from concourse import bass_utils

N_TOTAL = 500000
T = 20
BINS = 12
NCORES = 8
P = 128
SLOTS = T + 1

AOT = mybir.AluOpType
ACTF = mybir.ActivationFunctionType
F32 = mybir.dt.float32
I32 = mybir.dt.int32


def _steps_np():
    # bit-exact match of jnp: (arange(BINS) + 0.5) / BINS in f32
    return (np.arange(BINS, dtype=np.float32) + np.float32(0.5)) / np.float32(BINS)


def build_nc(beta: float, mq: float, n_pad: int, ft: int, ncores: int = NCORES,
             p: int = P):
    """Build the Bass module. n_pad = per-core padded sequence count
    (= p * f_core), ft = sequences per partition per tile."""
    f_core = n_pad // p
    assert f_core * p == n_pad and f_core % ft == 0
    ntiles = f_core // ft
    steps = _steps_np()

    nc = bacc.Bacc("TRN2", target_bir_lowering=False, debug=False,
                   enable_asserts=False, num_devices=ncores)

    d_perf = nc.dram_tensor("perf", [T, n_pad, 2], I32, kind="ExternalInput").ap()
    d_dobs = nc.dram_tensor("dobs", [T, n_pad], F32, kind="ExternalInput").ap()
    d_dpred = nc.dram_tensor("dpred", [n_pad], F32, kind="ExternalInput").ap()
    d_out = nc.dram_tensor("out", [p, f_core], F32, kind="ExternalOutput").ap()

    # Per-bin softplus-argument bias constants, registered as [128,1] const APs
    bias_vals = [float(np.float32(-np.float32(beta) * steps[k])) for k in range(BINS)]
    for v in dict.fromkeys(bias_vals):
        th = nc.alloc_sbuf_tensor(f"const-bias-{v!r}", [p, 1], F32)
        nc.gpsimd.memset(th.ap(), v)
        nc.const_aps.aps[(F32, v)] = th.ap()
    nc.all_engine_barrier()

    perf_v = d_perf.rearrange("t (p n) c -> p t n c", p=p)   # [p, T, f_core, 2]
    dobs_v = d_dobs.rearrange("t (p n) -> p t n", p=p)       # [p, T, f_core]

    with tile.TileContext(nc) as tc:
        with ExitStack() as ctx:
            inpool = ctx.enter_context(tc.tile_pool(name="in", bufs=3))
            work = ctx.enter_context(tc.tile_pool(name="work", bufs=2))
            keep = ctx.enter_context(tc.tile_pool(name="keep", bufs=1))

            C = keep.tile([p, f_core], F32, tag="C")
            DP = keep.tile([p, f_core], F32, tag="DP")
            nc.sync.dma_start(DP[:], d_dpred.rearrange("(p n) -> p n", p=p))

            for j in range(ntiles):
                sl = slice(j * ft, (j + 1) * ft)
                Pt = inpool.tile([p, T * 2 * ft], I32, tag="Pt")
                nc.sync.dma_start(
                    Pt[:].rearrange("p (t n c) -> p t n c", t=T, n=ft, c=2),
                    perf_v[:, :, sl, :])
                Dt = inpool.tile([p, T * ft], F32, tag="Dt")
                nc.sync.dma_start(
                    Dt[:].rearrange("p (t n) -> p t n", t=T), dobs_v[:, :, sl])

                # [p][n][t] iteration views
                Ptv = Pt[:].rearrange("p (t n c) -> p t n c", t=T, n=ft, c=2)
                p0v = Ptv[:, :, :, 0].rearrange("p t n -> p n t")
                p1v = Ptv[:, :, :, 1].rearrange("p t n -> p n t")
                Dv = Dt[:].rearrange("p (t n) -> p t n", t=T).rearrange(
                    "p t n -> p n t")

                LO = work.tile([p, ft * SLOTS], F32, tag="LO")
                HI = work.tile([p, ft * SLOTS], F32, tag="HI")
                LOv = LO[:].rearrange("p (n s) -> p n s", s=SLOTS)
                HIv = HI[:].rearrange("p (n s) -> p n s", s=SLOTS)
                lo_s = LOv[:, :, 0:T]
                hi_s = HIv[:, :, 0:T]

                # u = (p1 - 1) + d ; lo = (p0 * -1) + u ; hi = (p0 < 1) + d
                nc.vector.scalar_tensor_tensor(lo_s, p1v, 1.0, Dv,
                                               AOT.subtract, AOT.add)
                nc.vector.scalar_tensor_tensor(lo_s, p0v, -1.0, lo_s,
                                               AOT.mult, AOT.add)
                nc.vector.scalar_tensor_tensor(hi_s, p0v, 1.0, Dv,
                                               AOT.is_lt, AOT.add)
                nc.vector.memset(LOv[:, :, T:SLOTS], 0.0)
                nc.vector.memset(HIv[:, :, T:SLOTS], 0.0)

                CS = work.tile([p, ft * SLOTS], F32, tag="CS")
                nc.vector.tensor_tensor_scan(CS[:], LO[:], HI[:], 0.0,
                                             AOT.max, AOT.min)
                # c for this tile: scan state after step T-1
                cview = CS[:].rearrange("p (n s) -> p n s", s=SLOTS)[:, :, T - 1]
                nc.scalar.copy(C[:, sl], cview)

            # ---- phase B ----
            SP = keep.tile([p, BINS * f_core], F32, tag="SP")
            SPv = SP[:].rearrange("p (k n) -> p k n", k=BINS)
            for k in range(BINS):
                nc.scalar.activation(SPv[:, k, :], DP[:], ACTF.Exp,
                                     bias=bias_vals[k], scale=float(np.float32(beta)))
            nc.scalar.activation(SP[:], SP[:], ACTF.Ln, bias=1.0)
            nc.scalar.activation(SP[:], SP[:], ACTF.Exp, scale=float(np.float32(mq)))

            M = keep.tile([p, BINS * f_core], F32, tag="M")
            Mv = M[:].rearrange("p (k n) -> p k n", k=BINS)
            for k in range(BINS):
                nc.vector.tensor_scalar(Mv[:, k, :], C[:], float(steps[k]), None,
                                        AOT.is_le)
            nc.vector.tensor_tensor(SP[:], SP[:], M[:], AOT.mult)

            tsum = keep.tile([p, f_core], F32, tag="tsum")
            msum = keep.tile([p, f_core], F32, tag="msum")
            nc.vector.tensor_reduce(
                tsum[:], SP[:].rearrange("p (k n) -> p n k", k=BINS),
                mybir.AxisListType.X, AOT.add)
            nc.vector.tensor_reduce(
                msum[:], M[:].rearrange("p (k n) -> p n k", k=BINS),
                mybir.AxisListType.X, AOT.add)
            rec = keep.tile([p, f_core], F32, tag="rec")
            nc.vector.reciprocal(rec[:], msum[:])
            OUT = keep.tile([p, f_core], F32, tag="OUT")
            nc.vector.tensor_tensor(OUT[:], tsum[:], rec[:], AOT.mult)
            nc.sync.dma_start(d_out, OUT[:])

    nc.compile()
    return nc


_CACHE: dict = {}


def _get_nc(beta: float, mq: float):
    key = (beta, mq)
    if key not in _CACHE:
        _CACHE[key] = build_nc(beta, mq, n_pad=62720, ft=98)
    return _CACHE[key]


def make_in_maps(inptasksperf, difficulties_obs, difficulties_pred,
                 n_total=N_TOTAL, ncores=NCORES, n_pad=62720):
    """Host-side shard + pad. Returns list of per-core input dicts."""
    perf = np.asarray(inptasksperf, dtype=np.int32)
    dobs = np.asarray(difficulties_obs, dtype=np.float32)[..., 0]   # [T, N]
    dpred = np.asarray(difficulties_pred, dtype=np.float32)[..., 0]  # [N]
    nc_n = n_total // ncores
    in_maps = []
    for c in range(ncores):
        sl = slice(c * nc_n, (c + 1) * nc_n)
        pc = np.zeros((T, n_pad, 2), dtype=np.int32)
        pc[:, :nc_n, :] = perf[:, sl, :]
        dc = np.zeros((T, n_pad), dtype=np.float32)
        dc[:, :nc_n] = dobs[:, sl]
        dpc = np.zeros((n_pad,), dtype=np.float32)
        dpc[:nc_n] = dpred[sl]
        in_maps.append({"perf": pc, "dobs": dc, "dpred": dpc})
    return in_maps


def kernel(inptasksobs=None, inptasksperf=None, inptaskspred=None,
           num_obs_tasks=None, tasksobsids=None, taskspredids=None,
           difficulties_obs=None, difficulties_pred=None,
           betas=None, zetas=None, **_):
    beta = float(np.float32(np.asarray(betas).reshape(-1)[0]))
    zeta = np.float32(np.asarray(zetas).reshape(-1)[0])
    mq = float(np.float32(-(zeta * zeta)))

    nc = _get_nc(beta, mq)
    in_maps = make_in_maps(inptasksperf, difficulties_obs, difficulties_pred)
    res = bass_utils.run_bass_kernel_spmd(nc, in_maps,
                                          core_ids=list(range(NCORES)))
    nc_n = N_TOTAL // NCORES
    parts = [np.asarray(r["out"]).reshape(-1)[:nc_n] for r in res.results]
    return np.concatenate(parts).reshape(N_TOTAL, 1).astype(np.float32)


if __name__ == "__main__":
    # quick shape-only self-check
    rng = np.random.default_rng(0)
    ins = {
        "inptasksperf": rng.integers(0, 2, (T, N_TOTAL, 2)).astype(np.int32),
        "difficulties_obs": (0.9 * rng.random((T, N_TOTAL, 1))).astype(np.float32),
        "difficulties_pred": (0.9 * rng.random((N_TOTAL, 1))).astype(np.float32),
        "betas": np.array([7.0], np.float32),
        "zetas": np.array([0.5], np.float32),
    }
    out = kernel(**ins)
    print(out.shape, out.dtype, out[:5, 0])


# revision 2
# speedup vs baseline: 1.5088x; 1.5088x over previous
"""Trainium2 Bass kernel for nn_BidirectionalTrustModel (histogram_binning).

Computes, per observation sequence n (N = 500000, T = 20, BINS = 12):
  1. capability edge c[n]: sequential fold over t of
       c = max(c, d)  if perf==[0,1]
       c = min(c, d)  if perf[...,0]==1
       c              otherwise
  2. trust[n] = sum_k t_k * m_k / sum_k m_k  over 12 bin centers s_k,
       m_k = (c <= s_k),  t_k = (1 + exp(beta*(dpred - s_k)))**(-zeta^2)

Only inptasksperf, difficulties_obs, difficulties_pred, betas, zetas are used
(the other inputs are dead in the reference computation).

Device mapping (pure data parallel over 8 cores, no collectives):
  - per-core slice of 62500 sequences, padded to 62720 = 128 partitions x 490
  - host relayout to t-inner [128, 490, 20] planes; perf recoded losslessly
    to int8 planes w = p1 - p0 - 1 and p0 (cuts perf HBM 8B -> 2B per cell)
  - phase A (exact arithmetic: the fold's trigger value is exactly d because
    only +0.0 / *1.0 ever touch d):
      lo = d + w              in {d (max-step), d-1, d-2}
      hi = (p0 < 1) + d       in {d (min-step), d+1}
      slot-0 override lo0 = hi0 = d*m0 forces state = step0(0) regardless of
      scan carry-in (min(max(v, s), v) == v), so sequences pack back-to-back
      with NO reset slots and ONE contiguous tensor_tensor_scan(max, min)
      over [128, nseq*20] computes every capability edge.
  - phase B: per bin t_k via exp/ln/exp ACT chain (one act table), exact
    is_le masks (bf16), fused (C<=s_k)*t_k scalar_tensor_tensor, contiguous
    slab-add reductions, vector.reciprocal.
"""
import sys

if "/opt/trn_rl_repo" not in sys.path:
    sys.path.insert(0, "/opt/trn_rl_repo")

from contextlib import ExitStack

import numpy as np

import concourse.bacc as bacc
import concourse.bass as bass
import concourse.mybir as mybir
import concourse.tile as tile
from concourse import bass_utils

N_TOTAL = 500000
T = 20
BINS = 12
NCORES = 8
P = 128

AOT = mybir.AluOpType
ACTF = mybir.ActivationFunctionType
F32 = mybir.dt.float32
BF16 = mybir.dt.bfloat16
I8 = mybir.dt.int8


def _steps_np():
    # bit-exact match of jnp: (arange(BINS) + 0.5) / BINS in f32
    return (np.arange(BINS, dtype=np.float32) + np.float32(0.5)) / np.float32(BINS)


def build_nc(beta: float, mq: float, n_pad: int, ft: int, ncores: int = NCORES,
             p: int = P):
    """Build the Bass module. n_pad = per-core padded sequence count
    (= p * f_core), ft = sequences per partition per tile."""
    f_core = n_pad // p
    assert f_core * p == n_pad and f_core % ft == 0
    ntiles = f_core // ft
    steps = _steps_np()

    nc = bacc.Bacc("TRN2", target_bir_lowering=False, debug=False,
                   enable_asserts=False, num_devices=ncores)

    d_w = nc.dram_tensor("wplane", [p, f_core, T], I8, kind="ExternalInput").ap()
    d_p0 = nc.dram_tensor("p0plane", [p, f_core, T], I8, kind="ExternalInput").ap()
    d_d = nc.dram_tensor("dobs", [p, f_core, T], F32, kind="ExternalInput").ap()
    d_dpred = nc.dram_tensor("dpred", [n_pad], F32, kind="ExternalInput").ap()
    d_out = nc.dram_tensor("out", [p, f_core], F32, kind="ExternalOutput").ap()

    # Per-bin exp-argument bias constants, registered as [128,1] const APs
    bias_vals = [float(np.float32(-np.float32(beta) * steps[k])) for k in range(BINS)]
    for v in dict.fromkeys(bias_vals):
        th = nc.alloc_sbuf_tensor(f"const-bias-{v!r}", [p, 1], F32)
        nc.gpsimd.memset(th.ap(), v)
        nc.const_aps.aps[(F32, v)] = th.ap()
    nc.all_engine_barrier()

    FT20 = ft * T

    with tile.TileContext(nc) as tc:
        with ExitStack() as ctx:
            inpool = ctx.enter_context(tc.tile_pool(name="in", bufs=3))
            work = ctx.enter_context(tc.tile_pool(name="work", bufs=2))
            keep = ctx.enter_context(tc.tile_pool(name="keep", bufs=1))

            C = keep.tile([p, f_core], F32, tag="C")
            DP = keep.tile([p, f_core], F32, tag="DP")
            nc.sync.dma_start(DP[:], d_dpred.rearrange("(p n) -> p n", p=p))

            for j in range(ntiles):
                sl = slice(j * ft, (j + 1) * ft)
                Wt = inpool.tile([p, FT20], I8, tag="Wt")
                nc.sync.dma_start(
                    Wt[:].rearrange("p (n t) -> p n t", t=T), d_w[:, sl, :])
                P0t = inpool.tile([p, FT20], I8, tag="P0t")
                nc.sync.dma_start(
                    P0t[:].rearrange("p (n t) -> p n t", t=T), d_p0[:, sl, :])
                Dt = inpool.tile([p, FT20], F32, tag="Dt")
                nc.sync.dma_start(
                    Dt[:].rearrange("p (n t) -> p n t", t=T), d_d[:, sl, :])

                LO = work.tile([p, FT20], F32, tag="LO")
                HI = work.tile([p, FT20], F32, tag="HI")
                # lo = d + w ; hi = (p0 < 1) + d  (flat, fully contiguous)
                nc.vector.tensor_tensor(LO[:], Dt[:], Wt[:], AOT.add)
                nc.vector.scalar_tensor_tensor(HI[:], P0t[:], 1.0, Dt[:],
                                               AOT.is_lt, AOT.add)
                # slot-0 self-reset override: lo0 = hi0 = d0 * (w0 == 0)
                w0 = Wt[:].rearrange("p (n t) -> p n t", t=T)[:, :, 0]
                d0 = Dt[:].rearrange("p (n t) -> p n t", t=T)[:, :, 0]
                lo0 = LO[:].rearrange("p (n t) -> p n t", t=T)[:, :, 0]
                hi0 = HI[:].rearrange("p (n t) -> p n t", t=T)[:, :, 0]
                M0 = work.tile([p, ft], F32, tag="M0")
                nc.vector.tensor_scalar(M0[:], w0, 0, None, AOT.is_equal)
                nc.vector.tensor_tensor(lo0, M0[:], d0, AOT.mult)
                nc.scalar.copy(hi0, lo0)

                CS = work.tile([p, FT20], F32, tag="CS")
                nc.vector.tensor_tensor_scan(CS[:], LO[:], HI[:], 0.0,
                                             AOT.max, AOT.min)
                # c for this tile: scan state after step T-1
                cview = CS[:].rearrange("p (n t) -> p n t", t=T)[:, :, T - 1]
                nc.scalar.copy(C[:, sl], cview)

            # ---- phase B ----
            SP = keep.tile([p, BINS * f_core], F32, tag="SP")
            SPv = SP[:].rearrange("p (k n) -> p k n", k=BINS)
            for k in range(BINS):
                nc.scalar.activation(SPv[:, k, :], DP[:], ACTF.Exp,
                                     bias=bias_vals[k],
                                     scale=float(np.float32(beta)))
            nc.scalar.activation(SP[:], SP[:], ACTF.Ln, bias=1.0)
            nc.scalar.activation(SP[:], SP[:], ACTF.Exp,
                                 scale=float(np.float32(mq)))

            M = keep.tile([p, BINS * f_core], BF16, tag="M")
            Mv = M[:].rearrange("p (k n) -> p k n", k=BINS)
            for k in range(BINS):
                nc.vector.tensor_scalar(Mv[:, k, :], C[:], float(steps[k]),
                                        None, AOT.is_le)
                # masked t_k in place: SP_k = (C <= s_k) * SP_k
                nc.vector.scalar_tensor_tensor(SPv[:, k, :], C[:],
                                               float(steps[k]), SPv[:, k, :],
                                               AOT.is_le, AOT.mult)

            tsum = keep.tile([p, f_core], F32, tag="tsum")
            nc.vector.tensor_tensor(tsum[:], SPv[:, 0, :], SPv[:, 1, :], AOT.add)
            for k in range(2, BINS):
                nc.vector.tensor_tensor(tsum[:], tsum[:], SPv[:, k, :], AOT.add)
            msum = keep.tile([p, f_core], BF16, tag="msum")
            nc.vector.tensor_tensor(msum[:], Mv[:, 0, :], Mv[:, 1, :], AOT.add)
            for k in range(2, BINS):
                nc.vector.tensor_tensor(msum[:], msum[:], Mv[:, k, :], AOT.add)

            rec = keep.tile([p, f_core], F32, tag="rec")
            nc.vector.reciprocal(rec[:], msum[:])
            OUT = keep.tile([p, f_core], F32, tag="OUT")
            nc.vector.tensor_tensor(OUT[:], tsum[:], rec[:], AOT.mult)
            nc.sync.dma_start(d_out, OUT[:])

    nc.compile()
    return nc


_CACHE: dict = {}


def _get_nc(beta: float, mq: float):
    key = (beta, mq)
    if key not in _CACHE:
        _CACHE[key] = build_nc(beta, mq, n_pad=62720, ft=98)
    return _CACHE[key]


def make_in_maps(inptasksperf, difficulties_obs, difficulties_pred,
                 n_total=N_TOTAL, ncores=NCORES, n_pad=62720, p=P):
    """Host-side shard + pad + t-inner relayout + int8 recoding."""
    perf = np.asarray(inptasksperf)
    dobs = np.asarray(difficulties_obs, dtype=np.float32)[..., 0]    # [T, N]
    dpred = np.asarray(difficulties_pred, dtype=np.float32)[..., 0]  # [N]
    f_core = n_pad // p
    nc_n = n_total // ncores

    w_all = (perf[..., 1] - perf[..., 0] - 1).astype(np.int8)        # [T, N]
    p0_all = perf[..., 0].astype(np.int8)                            # [T, N]

    in_maps = []
    for c in range(ncores):
        sl = slice(c * nc_n, (c + 1) * nc_n)

        wpad = np.full((T, n_pad), -1, np.int8)
        wpad[:, :nc_n] = w_all[:, sl]
        wc = np.ascontiguousarray(wpad.reshape(T, p, f_core).transpose(1, 2, 0))

        p0pad = np.zeros((T, n_pad), np.int8)
        p0pad[:, :nc_n] = p0_all[:, sl]
        p0c = np.ascontiguousarray(p0pad.reshape(T, p, f_core).transpose(1, 2, 0))

        dpad = np.zeros((T, n_pad), np.float32)
        dpad[:, :nc_n] = dobs[:, sl]
        dc = np.ascontiguousarray(dpad.reshape(T, p, f_core).transpose(1, 2, 0))

        dpc = np.zeros((n_pad,), np.float32)
        dpc[:nc_n] = dpred[sl]
        in_maps.append({"wplane": wc, "p0plane": p0c, "dobs": dc, "dpred": dpc})
    return in_maps


def kernel(inptasksobs=None, inptasksperf=None, inptaskspred=None,
           num_obs_tasks=None, tasksobsids=None, taskspredids=None,
           difficulties_obs=None, difficulties_pred=None,
           betas=None, zetas=None, **_):
    beta = float(np.float32(np.asarray(betas).reshape(-1)[0]))
    zeta = np.float32(np.asarray(zetas).reshape(-1)[0])
    mq = float(np.float32(-(zeta * zeta)))

    nc = _get_nc(beta, mq)
    in_maps = make_in_maps(inptasksperf, difficulties_obs, difficulties_pred)
    res = bass_utils.run_bass_kernel_spmd(nc, in_maps,
                                          core_ids=list(range(NCORES)))
    nc_n = N_TOTAL // NCORES
    parts = [np.asarray(r["out"]).reshape(-1)[:nc_n] for r in res.results]
    return np.concatenate(parts).reshape(N_TOTAL, 1).astype(np.float32)


if __name__ == "__main__":
    rng = np.random.default_rng(0)
    ins = {
        "inptasksperf": rng.integers(0, 2, (T, N_TOTAL, 2)).astype(np.int32),
        "difficulties_obs": (0.9 * rng.random((T, N_TOTAL, 1))).astype(np.float32),
        "difficulties_pred": (0.9 * rng.random((N_TOTAL, 1))).astype(np.float32),
        "betas": np.array([7.0], np.float32),
        "zetas": np.array([0.5], np.float32),
    }
    out = kernel(**ins)
    print(out.shape, out.dtype, out[:5, 0])


# revision 3
# speedup vs baseline: 1.5353x; 1.0176x over previous
"""Trainium2 Bass kernel for nn_BidirectionalTrustModel (histogram_binning).

Computes, per observation sequence n (N = 500000, T = 20, BINS = 12):
  1. capability edge c[n]: sequential fold over t of
       c = max(c, d)  if perf==[0,1]
       c = min(c, d)  if perf[...,0]==1
       c              otherwise
  2. trust[n] = sum_k t_k * m_k / sum_k m_k  over 12 bin centers s_k,
       m_k = (c <= s_k),  t_k = (1 + exp(beta*(dpred - s_k)))**(-zeta^2)

Only inptasksperf, difficulties_obs, difficulties_pred, betas, zetas are used
(the other inputs are dead in the reference computation).

Device mapping (pure data parallel over 8 cores, no collectives):
  - per-core slice of 62500 sequences, padded to 62720 = 128 partitions x 490
  - host relayout to t-inner [128, 490, 20] planes; perf recoded losslessly
    to int8 planes w = p1 - p0 - 1 and p0 (cuts perf HBM 8B -> 2B per cell)
  - phase A (exact arithmetic: the fold's trigger value is exactly d because
    only +0.0 / *1.0 ever touch d):
      lo = d + w              in {d (max-step), d-1, d-2}
      hi = (p0 < 1) + d       in {d (min-step), d+1}
      slot-0 override lo0 = hi0 = d*m0 forces state = step0(0) regardless of
      scan carry-in (min(max(v, s), v) == v), so sequences pack back-to-back
      with NO reset slots and ONE contiguous tensor_tensor_scan(max, min)
      over [128, nseq*20] computes every capability edge.
  - phase B: per bin t_k via exp/ln/exp ACT chain (one act table), exact
    is_le masks (bf16), fused (C<=s_k)*t_k scalar_tensor_tensor, contiguous
    slab-add reductions, vector.reciprocal.
"""
import sys

if "/opt/trn_rl_repo" not in sys.path:
    sys.path.insert(0, "/opt/trn_rl_repo")

from contextlib import ExitStack

import numpy as np

import concourse.bacc as bacc
import concourse.bass as bass
import concourse.mybir as mybir
import concourse.tile as tile
from concourse import bass_utils

N_TOTAL = 500000
T = 20
BINS = 12
NCORES = 8
P = 128

AOT = mybir.AluOpType
ACTF = mybir.ActivationFunctionType
F32 = mybir.dt.float32
BF16 = mybir.dt.bfloat16
I8 = mybir.dt.int8


def _steps_np():
    # bit-exact match of jnp: (arange(BINS) + 0.5) / BINS in f32
    return (np.arange(BINS, dtype=np.float32) + np.float32(0.5)) / np.float32(BINS)


def build_nc(beta: float, mq: float, n_pad: int, ft: int, ncores: int = NCORES,
             p: int = P):
    """Build the Bass module. n_pad = per-core padded sequence count
    (= p * f_core), ft = sequences per partition per tile."""
    f_core = n_pad // p
    assert f_core * p == n_pad and f_core % ft == 0
    ntiles = f_core // ft
    steps = _steps_np()

    nc = bacc.Bacc("TRN2", target_bir_lowering=False, debug=False,
                   enable_asserts=False, num_devices=ncores)

    d_w = nc.dram_tensor("wplane", [p, f_core, T], I8, kind="ExternalInput").ap()
    d_p0 = nc.dram_tensor("p0plane", [p, f_core, T], I8, kind="ExternalInput").ap()
    d_d = nc.dram_tensor("dobs", [p, f_core, T], F32, kind="ExternalInput").ap()
    d_dpred = nc.dram_tensor("dpred", [n_pad], F32, kind="ExternalInput").ap()
    d_out = nc.dram_tensor("out", [p, f_core], F32, kind="ExternalOutput").ap()

    # Per-bin exp-argument bias constants, registered as [128,1] const APs
    bias_vals = [float(np.float32(-np.float32(beta) * steps[k])) for k in range(BINS)]
    for v in dict.fromkeys(bias_vals):
        th = nc.alloc_sbuf_tensor(f"const-bias-{v!r}", [p, 1], F32)
        nc.gpsimd.memset(th.ap(), v)
        nc.const_aps.aps[(F32, v)] = th.ap()
    nc.all_engine_barrier()

    FT20 = ft * T

    with tile.TileContext(nc) as tc:
        with ExitStack() as ctx:
            inpool = ctx.enter_context(tc.tile_pool(name="in", bufs=3))
            work = ctx.enter_context(tc.tile_pool(name="work", bufs=2))
            keep = ctx.enter_context(tc.tile_pool(name="keep", bufs=1))

            C = keep.tile([p, f_core], F32, tag="C")
            DP = keep.tile([p, f_core], F32, tag="DP")
            nc.sync.dma_start(DP[:], d_dpred.rearrange("(p n) -> p n", p=p))

            for j in range(ntiles):
                sl = slice(j * ft, (j + 1) * ft)
                Wt = inpool.tile([p, FT20], I8, tag="Wt")
                nc.sync.dma_start(
                    Wt[:].rearrange("p (n t) -> p n t", t=T), d_w[:, sl, :])
                P0t = inpool.tile([p, FT20], I8, tag="P0t")
                nc.sync.dma_start(
                    P0t[:].rearrange("p (n t) -> p n t", t=T), d_p0[:, sl, :])
                Dt = inpool.tile([p, FT20], F32, tag="Dt")
                nc.sync.dma_start(
                    Dt[:].rearrange("p (n t) -> p n t", t=T), d_d[:, sl, :])

                LO = work.tile([p, FT20], F32, tag="LO")
                HI = work.tile([p, FT20], F32, tag="HI")
                # lo = d + w ; hi = (p0 < 1) + d  (flat, fully contiguous)
                nc.vector.tensor_tensor(LO[:], Dt[:], Wt[:], AOT.add)
                nc.vector.scalar_tensor_tensor(HI[:], P0t[:], 1.0, Dt[:],
                                               AOT.is_lt, AOT.add)
                # slot-0 self-reset override: lo0 = hi0 = d0 * (w0 == 0)
                w0 = Wt[:].rearrange("p (n t) -> p n t", t=T)[:, :, 0]
                d0 = Dt[:].rearrange("p (n t) -> p n t", t=T)[:, :, 0]
                lo0 = LO[:].rearrange("p (n t) -> p n t", t=T)[:, :, 0]
                hi0 = HI[:].rearrange("p (n t) -> p n t", t=T)[:, :, 0]
                M0 = work.tile([p, ft], F32, tag="M0")
                nc.vector.tensor_scalar(M0[:], w0, 0, None, AOT.is_equal)
                nc.vector.tensor_tensor(lo0, M0[:], d0, AOT.mult)
                nc.scalar.copy(hi0, lo0)

                CS = work.tile([p, FT20], F32, tag="CS")
                nc.vector.tensor_tensor_scan(CS[:], LO[:], HI[:], 0.0,
                                             AOT.max, AOT.min)
                # c for this tile: scan state after step T-1
                cview = CS[:].rearrange("p (n t) -> p n t", t=T)[:, :, T - 1]
                nc.scalar.copy(C[:, sl], cview)

            # ---- phase B ----
            SP = keep.tile([p, BINS * f_core], F32, tag="SP")
            SPv = SP[:].rearrange("p (k n) -> p k n", k=BINS)
            for k in range(BINS):
                nc.scalar.activation(SPv[:, k, :], DP[:], ACTF.Exp,
                                     bias=bias_vals[k],
                                     scale=float(np.float32(beta)))
            nc.scalar.activation(SP[:], SP[:], ACTF.Ln, bias=1.0)
            nc.scalar.activation(SP[:], SP[:], ACTF.Exp,
                                 scale=float(np.float32(mq)))

            M = keep.tile([p, BINS * f_core], BF16, tag="M")
            Mv = M[:].rearrange("p (k n) -> p k n", k=BINS)
            for k in range(BINS):
                nc.vector.tensor_scalar(Mv[:, k, :], C[:], float(steps[k]),
                                        None, AOT.is_le)
            # masked t_k in place, one big op: SP = SP * M
            nc.vector.tensor_tensor(SP[:], SP[:], M[:], AOT.mult)

            def pair_tree(src_v, nbins, dtype, tag):
                """Pairwise-tree sum over the k axis of a [p, k, n] view."""
                v = src_v
                nk = nbins
                lvl = 0
                while nk > 1:
                    half = nk // 2
                    out_t = keep.tile([p, (half + nk % 2) * f_core], dtype,
                                      tag=f"{tag}L{lvl}")
                    ov = out_t[:].rearrange("p (k n) -> p k n", k=half + nk % 2)
                    nc.vector.tensor_tensor(ov[:, 0:half, :],
                                            v[:, 0:2 * half:2, :],
                                            v[:, 1:2 * half:2, :], AOT.add)
                    if nk % 2:
                        nc.scalar.copy(ov[:, half, :], v[:, nk - 1, :])
                    v, nk, lvl = ov, half + nk % 2, lvl + 1
                return v[:, 0, :]

            tsum = pair_tree(SPv, BINS, F32, "ts")
            msum = pair_tree(Mv, BINS, BF16, "ms")

            # 1/msum on ACT: exp(-ln(msum)) — exact ints 1..12, ~ulp error
            rec = keep.tile([p, f_core], F32, tag="rec")
            nc.scalar.activation(rec[:], msum, ACTF.Ln)
            nc.scalar.activation(rec[:], rec[:], ACTF.Exp, scale=-1.0)
            OUT = keep.tile([p, f_core], F32, tag="OUT")
            nc.vector.tensor_tensor(OUT[:], tsum, rec[:], AOT.mult)
            nc.sync.dma_start(d_out, OUT[:])

    nc.compile()
    return nc


_CACHE: dict = {}


def _get_nc(beta: float, mq: float):
    key = (beta, mq)
    if key not in _CACHE:
        _CACHE[key] = build_nc(beta, mq, n_pad=62720, ft=98)
    return _CACHE[key]


def make_in_maps(inptasksperf, difficulties_obs, difficulties_pred,
                 n_total=N_TOTAL, ncores=NCORES, n_pad=62720, p=P):
    """Host-side shard + pad + t-inner relayout + int8 recoding."""
    perf = np.asarray(inptasksperf)
    dobs = np.asarray(difficulties_obs, dtype=np.float32)[..., 0]    # [T, N]
    dpred = np.asarray(difficulties_pred, dtype=np.float32)[..., 0]  # [N]
    f_core = n_pad // p
    nc_n = n_total // ncores

    w_all = (perf[..., 1] - perf[..., 0] - 1).astype(np.int8)        # [T, N]
    p0_all = perf[..., 0].astype(np.int8)                            # [T, N]

    in_maps = []
    for c in range(ncores):
        sl = slice(c * nc_n, (c + 1) * nc_n)

        wpad = np.full((T, n_pad), -1, np.int8)
        wpad[:, :nc_n] = w_all[:, sl]
        wc = np.ascontiguousarray(wpad.reshape(T, p, f_core).transpose(1, 2, 0))

        p0pad = np.zeros((T, n_pad), np.int8)
        p0pad[:, :nc_n] = p0_all[:, sl]
        p0c = np.ascontiguousarray(p0pad.reshape(T, p, f_core).transpose(1, 2, 0))

        dpad = np.zeros((T, n_pad), np.float32)
        dpad[:, :nc_n] = dobs[:, sl]
        dc = np.ascontiguousarray(dpad.reshape(T, p, f_core).transpose(1, 2, 0))

        dpc = np.zeros((n_pad,), np.float32)
        dpc[:nc_n] = dpred[sl]
        in_maps.append({"wplane": wc, "p0plane": p0c, "dobs": dc, "dpred": dpc})
    return in_maps


def kernel(inptasksobs=None, inptasksperf=None, inptaskspred=None,
           num_obs_tasks=None, tasksobsids=None, taskspredids=None,
           difficulties_obs=None, difficulties_pred=None,
           betas=None, zetas=None, **_):
    beta = float(np.float32(np.asarray(betas).reshape(-1)[0]))
    zeta = np.float32(np.asarray(zetas).reshape(-1)[0])
    mq = float(np.float32(-(zeta * zeta)))

    nc = _get_nc(beta, mq)
    in_maps = make_in_maps(inptasksperf, difficulties_obs, difficulties_pred)
    res = bass_utils.run_bass_kernel_spmd(nc, in_maps,
                                          core_ids=list(range(NCORES)))
    nc_n = N_TOTAL // NCORES
    parts = [np.asarray(r["out"]).reshape(-1)[:nc_n] for r in res.results]
    return np.concatenate(parts).reshape(N_TOTAL, 1).astype(np.float32)


if __name__ == "__main__":
    rng = np.random.default_rng(0)
    ins = {
        "inptasksperf": rng.integers(0, 2, (T, N_TOTAL, 2)).astype(np.int32),
        "difficulties_obs": (0.9 * rng.random((T, N_TOTAL, 1))).astype(np.float32),
        "difficulties_pred": (0.9 * rng.random((N_TOTAL, 1))).astype(np.float32),
        "betas": np.array([7.0], np.float32),
        "zetas": np.array([0.5], np.float32),
    }
    out = kernel(**ins)
    print(out.shape, out.dtype, out[:5, 0])


# revision 10
# speedup vs baseline: 1.5469x; 1.0075x over previous
"""Trainium2 Bass kernel for nn_BidirectionalTrustModel (histogram_binning).

Computes, per observation sequence n (N = 500000, T = 20, BINS = 12):
  1. capability edge c[n]: sequential fold over t of
       c = max(c, d)  if perf==[0,1]
       c = min(c, d)  if perf[...,0]==1
       c              otherwise
  2. trust[n] = sum_k t_k * m_k / sum_k m_k  over 12 bin centers s_k,
       m_k = (c <= s_k),  t_k = (1 + exp(beta*(dpred - s_k)))**(-zeta^2)

Only inptasksperf, difficulties_obs, difficulties_pred, betas, zetas are used
(the other inputs are dead in the reference computation).

Device mapping (pure data parallel over 8 cores, no collectives):
  - per-core slice of 62500 sequences, padded to 62720 = 128 partitions x 490
  - host relayout to t-inner [128, 490, 20] planes; perf recoded losslessly
    to int8 planes w = p1 - p0 - 1 and p0 (cuts perf HBM 8B -> 2B per cell)
  - phase A (exact arithmetic: the fold's trigger value is exactly d because
    only +0.0 / *1.0 ever touch d):
      lo = d + w              in {d (max-step), d-1, d-2}
      hi = (p0 < 1) + d       in {d (min-step), d+1}
      slot-0 override lo0 = hi0 = d*m0 forces state = step0(0) regardless of
      scan carry-in (min(max(v, s), v) == v), so sequences pack back-to-back
      with NO reset slots and ONE contiguous tensor_tensor_scan(max, min)
      over [128, nseq*20] computes every capability edge.
  - phase B: per bin t_k via exp/ln/exp ACT chain (one act table), exact
    is_le masks (bf16), fused (C<=s_k)*t_k scalar_tensor_tensor, contiguous
    slab-add reductions, vector.reciprocal.
"""
import sys

if "/opt/trn_rl_repo" not in sys.path:
    sys.path.insert(0, "/opt/trn_rl_repo")

from contextlib import ExitStack

import numpy as np

import concourse.bacc as bacc
import concourse.bass as bass
import concourse.mybir as mybir
import concourse.tile as tile
from concourse import bass_utils

N_TOTAL = 500000
T = 20
BINS = 12
NCORES = 8
P = 128

AOT = mybir.AluOpType
ACTF = mybir.ActivationFunctionType
F32 = mybir.dt.float32
BF16 = mybir.dt.bfloat16
I8 = mybir.dt.int8


def _steps_np():
    # bit-exact match of jnp: (arange(BINS) + 0.5) / BINS in f32
    return (np.arange(BINS, dtype=np.float32) + np.float32(0.5)) / np.float32(BINS)


def build_nc(beta: float, mq: float, n_pad: int, ft, ncores: int = NCORES,
             p: int = P):
    """Build the Bass module. n_pad = per-core padded sequence count
    (= p * f_core), ft = tile widths (int or list, sequences per partition
    per tile; first tiles smaller to prime the DMA/compute pipeline)."""
    f_core = n_pad // p
    assert f_core * p == n_pad
    if isinstance(ft, int):
        assert f_core % ft == 0
        fts = [ft] * (f_core // ft)
    else:
        fts = list(ft)
        assert sum(fts) == f_core
    steps = _steps_np()

    nc = bacc.Bacc("TRN2", target_bir_lowering=False, debug=False,
                   enable_asserts=False, num_devices=ncores)

    d_w = nc.dram_tensor("wplane", [p, f_core, T], I8, kind="ExternalInput").ap()
    d_p0 = nc.dram_tensor("p0plane", [p, f_core, T], I8, kind="ExternalInput").ap()
    d_d = nc.dram_tensor("dobs", [p, f_core, T], F32, kind="ExternalInput").ap()
    d_dpred = nc.dram_tensor("dpred", [n_pad], F32, kind="ExternalInput").ap()
    d_out = nc.dram_tensor("out", [p, f_core], F32, kind="ExternalOutput").ap()

    # Per-bin exp-argument bias constants, registered as [128,1] const APs
    bias_vals = [float(np.float32(-np.float32(beta) * steps[k])) for k in range(BINS)]
    for v in dict.fromkeys(bias_vals):
        th = nc.alloc_sbuf_tensor(f"const-bias-{v!r}", [p, 1], F32)
        nc.gpsimd.memset(th.ap(), v)
        nc.const_aps.aps[(F32, v)] = th.ap()
    nc.all_engine_barrier()

    with tile.TileContext(nc) as tc:
        with ExitStack() as ctx:
            inpool = ctx.enter_context(tc.tile_pool(name="in", bufs=len(fts)))
            work = ctx.enter_context(tc.tile_pool(name="work", bufs=2))
            keep = ctx.enter_context(tc.tile_pool(name="keep", bufs=1))

            C = keep.tile([p, f_core], F32, tag="C")
            DP = keep.tile([p, f_core], F32, tag="DP")
            nc.sync.dma_start(DP[:], d_dpred.rearrange("(p n) -> p n", p=p))

            base = 0
            for j, ftj in enumerate(fts):
                sl = slice(base, base + ftj)
                base += ftj
                FT20 = ftj * T
                Dt = inpool.tile([p, FT20], F32, tag="Dt")
                nc.sync.dma_start(
                    Dt[:].rearrange("p (n t) -> p n t", t=T), d_d[:, sl, :])
                Wt = inpool.tile([p, FT20], I8, tag="Wt")
                nc.sync.dma_start(
                    Wt[:].rearrange("p (n t) -> p n t", t=T), d_w[:, sl, :])
                P0t = inpool.tile([p, FT20], I8, tag="P0t")
                nc.sync.dma_start(
                    P0t[:].rearrange("p (n t) -> p n t", t=T), d_p0[:, sl, :])

                LO = work.tile([p, FT20], F32, tag="LO")
                HI = work.tile([p, FT20], F32, tag="HI")
                # lo = d + w ; hi = (p0 < 1) + d  (flat, fully contiguous)
                nc.vector.tensor_tensor(LO[:], Dt[:], Wt[:], AOT.add)
                nc.vector.scalar_tensor_tensor(HI[:], P0t[:], 1.0, Dt[:],
                                               AOT.is_lt, AOT.add)
                # slot-0 self-reset override: lo0 = hi0 = d0 * (w0 == 0)
                w0 = Wt[:].rearrange("p (n t) -> p n t", t=T)[:, :, 0]
                d0 = Dt[:].rearrange("p (n t) -> p n t", t=T)[:, :, 0]
                lo0 = LO[:].rearrange("p (n t) -> p n t", t=T)[:, :, 0]
                hi0 = HI[:].rearrange("p (n t) -> p n t", t=T)[:, :, 0]
                M0 = work.tile([p, ftj], F32, tag="M0")
                nc.vector.tensor_scalar(M0[:], w0, 0, None, AOT.is_equal)
                nc.vector.tensor_tensor(lo0, M0[:], d0, AOT.mult)
                nc.scalar.copy(hi0, lo0)

                CS = work.tile([p, FT20], F32, tag="CS")
                nc.vector.tensor_tensor_scan(CS[:], LO[:], HI[:], 0.0,
                                             AOT.max, AOT.min)
                # c for this tile: scan state after step T-1
                cview = CS[:].rearrange("p (n t) -> p n t", t=T)[:, :, T - 1]
                nc.scalar.copy(C[:, sl], cview)

            # ---- phase B ----
            SP = keep.tile([p, BINS * f_core], F32, tag="SP")
            SPv = SP[:].rearrange("p (k n) -> p k n", k=BINS)
            for k in range(BINS):
                nc.scalar.activation(SPv[:, k, :], DP[:], ACTF.Exp,
                                     bias=bias_vals[k],
                                     scale=float(np.float32(beta)))
            nc.scalar.activation(SP[:], SP[:], ACTF.Ln, bias=1.0)
            nc.scalar.activation(SP[:], SP[:], ACTF.Exp,
                                 scale=float(np.float32(mq)))

            M = keep.tile([p, BINS * f_core], BF16, tag="M")
            Mv = M[:].rearrange("p (k n) -> p k n", k=BINS)
            for k in range(BINS):
                nc.vector.tensor_scalar(Mv[:, k, :], C[:], float(steps[k]),
                                        None, AOT.is_le)

            def pair_tree(src_v, nbins, dtype, tag):
                """Pairwise-tree sum over the k axis of a [p, k, n] view."""
                v = src_v
                nk = nbins
                lvl = 0
                while nk > 1:
                    half = nk // 2
                    out_t = keep.tile([p, (half + nk % 2) * f_core], dtype,
                                      tag=f"{tag}L{lvl}")
                    ov = out_t[:].rearrange("p (k n) -> p k n", k=half + nk % 2)
                    nc.vector.tensor_tensor(ov[:, 0:half, :],
                                            v[:, 0:2 * half:2, :],
                                            v[:, 1:2 * half:2, :], AOT.add)
                    if nk % 2:
                        nc.scalar.copy(ov[:, half, :], v[:, nk - 1, :])
                    v, nk, lvl = ov, half + nk % 2, lvl + 1
                return v[:, 0, :]

            # msum path first so ACT's reciprocal overlaps the tsum work
            msum = pair_tree(Mv, BINS, BF16, "ms")
            # 1/msum on ACT: exp(-ln(msum)) — exact ints 1..12, ~ulp error
            rec = keep.tile([p, f_core], F32, tag="rec")
            nc.scalar.activation(rec[:], msum, ACTF.Ln)
            nc.scalar.activation(rec[:], rec[:], ACTF.Exp, scale=-1.0)

            # masked t_k in place, one big op: SP = SP * M
            nc.vector.tensor_tensor(SP[:], SP[:], M[:], AOT.mult)
            tsum = pair_tree(SPv, BINS, F32, "ts")
            OUT = keep.tile([p, f_core], F32, tag="OUT")
            nc.vector.tensor_tensor(OUT[:], tsum, rec[:], AOT.mult)
            nc.sync.dma_start(d_out, OUT[:])

    nc.compile()
    return nc


_CACHE: dict = {}


def _get_nc(beta: float, mq: float):
    key = (beta, mq)
    if key not in _CACHE:
        _CACHE[key] = build_nc(beta, mq, n_pad=62720,
                               ft=[28, 70, 98, 98, 98, 98])
    return _CACHE[key]


def make_in_maps(inptasksperf, difficulties_obs, difficulties_pred,
                 n_total=N_TOTAL, ncores=NCORES, n_pad=62720, p=P):
    """Host-side shard + pad + t-inner relayout + int8 recoding."""
    perf = np.asarray(inptasksperf)
    dobs = np.asarray(difficulties_obs, dtype=np.float32)[..., 0]    # [T, N]
    dpred = np.asarray(difficulties_pred, dtype=np.float32)[..., 0]  # [N]
    f_core = n_pad // p
    nc_n = n_total // ncores

    w_all = (perf[..., 1] - perf[..., 0] - 1).astype(np.int8)        # [T, N]
    p0_all = perf[..., 0].astype(np.int8)                            # [T, N]

    in_maps = []
    for c in range(ncores):
        sl = slice(c * nc_n, (c + 1) * nc_n)

        wpad = np.full((T, n_pad), -1, np.int8)
        wpad[:, :nc_n] = w_all[:, sl]
        wc = np.ascontiguousarray(wpad.reshape(T, p, f_core).transpose(1, 2, 0))

        p0pad = np.zeros((T, n_pad), np.int8)
        p0pad[:, :nc_n] = p0_all[:, sl]
        p0c = np.ascontiguousarray(p0pad.reshape(T, p, f_core).transpose(1, 2, 0))

        dpad = np.zeros((T, n_pad), np.float32)
        dpad[:, :nc_n] = dobs[:, sl]
        dc = np.ascontiguousarray(dpad.reshape(T, p, f_core).transpose(1, 2, 0))

        dpc = np.zeros((n_pad,), np.float32)
        dpc[:nc_n] = dpred[sl]
        in_maps.append({"wplane": wc, "p0plane": p0c, "dobs": dc, "dpred": dpc})
    return in_maps


def kernel(inptasksobs=None, inptasksperf=None, inptaskspred=None,
           num_obs_tasks=None, tasksobsids=None, taskspredids=None,
           difficulties_obs=None, difficulties_pred=None,
           betas=None, zetas=None, **_):
    beta = float(np.float32(np.asarray(betas).reshape(-1)[0]))
    zeta = np.float32(np.asarray(zetas).reshape(-1)[0])
    mq = float(np.float32(-(zeta * zeta)))

    nc = _get_nc(beta, mq)
    in_maps = make_in_maps(inptasksperf, difficulties_obs, difficulties_pred)
    res = bass_utils.run_bass_kernel_spmd(nc, in_maps,
                                          core_ids=list(range(NCORES)))
    nc_n = N_TOTAL // NCORES
    parts = [np.asarray(r["out"]).reshape(-1)[:nc_n] for r in res.results]
    return np.concatenate(parts).reshape(N_TOTAL, 1).astype(np.float32)


if __name__ == "__main__":
    rng = np.random.default_rng(0)
    ins = {
        "inptasksperf": rng.integers(0, 2, (T, N_TOTAL, 2)).astype(np.int32),
        "difficulties_obs": (0.9 * rng.random((T, N_TOTAL, 1))).astype(np.float32),
        "difficulties_pred": (0.9 * rng.random((N_TOTAL, 1))).astype(np.float32),
        "betas": np.array([7.0], np.float32),
        "zetas": np.array([0.5], np.float32),
    }
    out = kernel(**ins)
    print(out.shape, out.dtype, out[:5, 0])


# revision 17
# speedup vs baseline: 1.6029x; 1.0362x over previous
"""Trainium2 Bass kernel for nn_BidirectionalTrustModel (histogram_binning).

Computes, per observation sequence n (N = 500000, T = 20, BINS = 12):
  1. capability edge c[n]: sequential fold over t of
       c = max(c, d)  if perf==[0,1]
       c = min(c, d)  if perf[...,0]==1
       c              otherwise
  2. trust[n] = sum_k t_k * m_k / sum_k m_k  over 12 bin centers s_k,
       m_k = (c <= s_k),  t_k = (1 + exp(beta*(dpred - s_k)))**(-zeta^2)

Only inptasksperf, difficulties_obs, difficulties_pred, betas, zetas are used
(the other inputs are dead in the reference computation).

Device mapping (pure data parallel over 8 cores, no collectives):
  - per-core slice of 62500 sequences, padded to 62720 = 128 partitions x 490
  - host relayout to t-inner [128, 490, 20] planes; perf recoded losslessly
    to int8 planes w = p1 - p0 - 1 and p0 (cuts perf HBM 8B -> 2B per cell)
  - phase A (exact arithmetic: the fold's trigger value is exactly d because
    only +0.0 / *1.0 ever touch d):
      lo = d + w              in {d (max-step), d-1, d-2}
      hi = (p0 < 1) + d       in {d (min-step), d+1}
      slot-0 override lo0 = hi0 = d*m0 forces state = step0(0) regardless of
      scan carry-in (min(max(v, s), v) == v), so sequences pack back-to-back
      with NO reset slots and ONE contiguous tensor_tensor_scan(max, min)
      over [128, nseq*20] computes every capability edge.
  - phase B: per bin t_k via exp/ln/exp ACT chain (one act table), exact
    is_le masks (bf16), fused (C<=s_k)*t_k scalar_tensor_tensor, contiguous
    slab-add reductions, vector.reciprocal.
"""
import sys

if "/opt/trn_rl_repo" not in sys.path:
    sys.path.insert(0, "/opt/trn_rl_repo")

from contextlib import ExitStack

import numpy as np

import concourse.bacc as bacc
import concourse.bass as bass
import concourse.mybir as mybir
import concourse.tile as tile
from concourse import bass_utils
from concourse.hw_specs import get_activation_tables as _orig_act_tables


def _combined_act_tables(arch):
    """Keep only natural_log_exp_and_others usable (positions preserved —
    the list index is the act_func_set_id) so Exp/Ln/Copy all resolve to ONE
    table: no ACT_TABLE_LOAD thrash between exp and ln."""
    t = _orig_act_tables(arch)
    return {k: (v if k == "natural_log_exp_and_others" else set())
            for k, v in t.items()}


bacc.get_activation_tables = _combined_act_tables

N_TOTAL = 500000
T = 20
BINS = 12
NCORES = 8
P = 128

AOT = mybir.AluOpType
ACTF = mybir.ActivationFunctionType
F32 = mybir.dt.float32
BF16 = mybir.dt.bfloat16
I8 = mybir.dt.int8


def _steps_np():
    # bit-exact match of jnp: (arange(BINS) + 0.5) / BINS in f32
    return (np.arange(BINS, dtype=np.float32) + np.float32(0.5)) / np.float32(BINS)


def build_nc(beta: float, mq: float, n_pad: int, ft, ncores: int = NCORES,
             p: int = P):
    """Build the Bass module. n_pad = per-core padded sequence count
    (= p * f_core), ft = tile widths (int or list, sequences per partition
    per tile; first tiles smaller to prime the DMA/compute pipeline)."""
    f_core = n_pad // p
    assert f_core * p == n_pad
    if isinstance(ft, int):
        assert f_core % ft == 0
        fts = [ft] * (f_core // ft)
    else:
        fts = list(ft)
        assert sum(fts) == f_core
    steps = _steps_np()

    nc = bacc.Bacc("TRN2", target_bir_lowering=False, debug=False,
                   enable_asserts=False, num_devices=ncores)

    d_wp = nc.dram_tensor("wp", [p, 2, f_core, T], I8, kind="ExternalInput").ap()
    d_d = nc.dram_tensor("dobs", [p, f_core, T], F32, kind="ExternalInput").ap()
    d_dpred = nc.dram_tensor("dpred", [n_pad], F32, kind="ExternalInput").ap()
    d_consts = nc.dram_tensor("consts", [p, BINS], F32,
                              kind="ExternalInput").ap()
    d_out = nc.dram_tensor("out", [p, f_core], F32, kind="ExternalOutput").ap()

    with tile.TileContext(nc) as tc:
        with ExitStack() as ctx:
            inpool = ctx.enter_context(tc.tile_pool(name="in", bufs=len(fts)))
            work = ctx.enter_context(tc.tile_pool(name="work", bufs=2))
            keep = ctx.enter_context(tc.tile_pool(name="keep", bufs=1))

            C = keep.tile([p, f_core], F32, tag="C")
            DP = keep.tile([p, f_core], F32, tag="DP")
            nc.scalar.dma_start(DP[:], d_dpred.rearrange("(p n) -> p n", p=p))
            CB = keep.tile([p, BINS], F32, tag="CB")
            nc.scalar.dma_start(CB[:], d_consts)

            base = 0
            for j, ftj in enumerate(fts):
                sl = slice(base, base + ftj)
                base += ftj
                FT20 = ftj * T
                Dt = inpool.tile([p, FT20], F32, tag="Dt")
                nc.sync.dma_start(
                    Dt[:].rearrange("p (n t) -> p n t", t=T), d_d[:, sl, :])
                WPt = inpool.tile([p, 2 * FT20], I8, tag="WPt")
                nc.scalar.dma_start(
                    WPt[:].rearrange("p (c n t) -> p c n t", c=2, t=T),
                    d_wp[:, :, sl, :])
                Wt = WPt[:, 0:FT20]
                P0t = WPt[:, FT20:2 * FT20]

                LO = work.tile([p, FT20], F32, tag="LO")
                HI = work.tile([p, FT20], F32, tag="HI")
                # lo = d + w ; hi = (p0 < 1) + d  (flat, fully contiguous)
                nc.vector.tensor_tensor(LO[:], Dt[:], Wt, AOT.add)
                nc.vector.scalar_tensor_tensor(HI[:], P0t, 1.0, Dt[:],
                                               AOT.is_lt, AOT.add)
                # slot-0 self-reset override: lo0 = hi0 = d0 * (w0 == 0)
                w0 = Wt.rearrange("p (n t) -> p n t", t=T)[:, :, 0]
                d0 = Dt[:].rearrange("p (n t) -> p n t", t=T)[:, :, 0]
                lo0 = LO[:].rearrange("p (n t) -> p n t", t=T)[:, :, 0]
                hi0 = HI[:].rearrange("p (n t) -> p n t", t=T)[:, :, 0]
                M0 = work.tile([p, ftj], F32, tag="M0")
                nc.vector.tensor_scalar(M0[:], w0, 0, None, AOT.is_equal)
                nc.vector.tensor_tensor(lo0, M0[:], d0, AOT.mult)
                nc.scalar.copy(hi0, lo0)

                CS = work.tile([p, FT20], F32, tag="CS")
                nc.vector.tensor_tensor_scan(CS[:], LO[:], HI[:], 0.0,
                                             AOT.max, AOT.min)
                # c for this tile: scan state after step T-1
                cview = CS[:].rearrange("p (n t) -> p n t", t=T)[:, :, T - 1]
                nc.scalar.copy(C[:, sl], cview)

            # ---- phase B ----
            SP = keep.tile([p, BINS * f_core], F32, tag="SP")
            SPv = SP[:].rearrange("p (k n) -> p k n", k=BINS)
            for k in range(BINS):
                nc.scalar.activation(SPv[:, k, :], DP[:], ACTF.Exp,
                                     bias=CB[:, k:k + 1],
                                     scale=float(np.float32(beta)))
            nc.scalar.activation(SP[:], SP[:], ACTF.Ln, bias=1.0)
            nc.scalar.activation(SP[:], SP[:], ACTF.Exp,
                                 scale=float(np.float32(mq)))

            M = keep.tile([p, BINS * f_core], BF16, tag="M")
            Mv = M[:].rearrange("p (k n) -> p k n", k=BINS)
            for k in range(BINS):
                nc.vector.tensor_scalar(Mv[:, k, :], C[:], float(steps[k]),
                                        None, AOT.is_le)

            def pair_tree(src_v, nbins, dtype, tag):
                """Pairwise-tree sum over the k axis of a [p, k, n] view."""
                v = src_v
                nk = nbins
                lvl = 0
                while nk > 1:
                    half = nk // 2
                    out_t = keep.tile([p, (half + nk % 2) * f_core], dtype,
                                      tag=f"{tag}L{lvl}")
                    ov = out_t[:].rearrange("p (k n) -> p k n", k=half + nk % 2)
                    nc.vector.tensor_tensor(ov[:, 0:half, :],
                                            v[:, 0:2 * half:2, :],
                                            v[:, 1:2 * half:2, :], AOT.add)
                    if nk % 2:
                        nc.scalar.copy(ov[:, half, :], v[:, nk - 1, :])
                    v, nk, lvl = ov, half + nk % 2, lvl + 1
                return v[:, 0, :]

            # msum path first so ACT's reciprocal overlaps the tsum work
            msum = pair_tree(Mv, BINS, BF16, "ms")
            # 1/msum on ACT: exp(-ln(msum)) — exact ints 1..12, ~ulp error
            rec = keep.tile([p, f_core], F32, tag="rec")
            nc.scalar.activation(rec[:], msum, ACTF.Ln)
            nc.scalar.activation(rec[:], rec[:], ACTF.Exp, scale=-1.0)

            # masked t_k in place, one big op: SP = SP * M
            nc.vector.tensor_tensor(SP[:], SP[:], M[:], AOT.mult)
            tsum = pair_tree(SPv, BINS, F32, "ts")
            OUT = keep.tile([p, f_core], F32, tag="OUT")
            nc.vector.tensor_tensor(OUT[:], tsum, rec[:], AOT.mult)
            nc.sync.dma_start(d_out, OUT[:])

    nc.compile()
    return nc


_CACHE: dict = {}


def _get_nc(beta: float, mq: float):
    key = (beta, mq)
    if key not in _CACHE:
        _CACHE[key] = build_nc(beta, mq, n_pad=62720,
                               ft=[28, 70, 98, 98, 98, 98])
    return _CACHE[key]


def make_in_maps(inptasksperf, difficulties_obs, difficulties_pred,
                 n_total=N_TOTAL, ncores=NCORES, n_pad=62720, p=P):
    """Host-side shard + pad + t-inner relayout + int8 recoding."""
    perf = np.asarray(inptasksperf)
    dobs = np.asarray(difficulties_obs, dtype=np.float32)[..., 0]    # [T, N]
    dpred = np.asarray(difficulties_pred, dtype=np.float32)[..., 0]  # [N]
    f_core = n_pad // p
    nc_n = n_total // ncores

    w_all = (perf[..., 1] - perf[..., 0] - 1).astype(np.int8)        # [T, N]
    p0_all = perf[..., 0].astype(np.int8)                            # [T, N]

    in_maps = []
    for c in range(ncores):
        sl = slice(c * nc_n, (c + 1) * nc_n)

        wpad = np.full((T, n_pad), -1, np.int8)
        wpad[:, :nc_n] = w_all[:, sl]
        wc = wpad.reshape(T, p, f_core).transpose(1, 2, 0)

        p0pad = np.zeros((T, n_pad), np.int8)
        p0pad[:, :nc_n] = p0_all[:, sl]
        p0c = p0pad.reshape(T, p, f_core).transpose(1, 2, 0)

        wp = np.ascontiguousarray(np.stack([wc, p0c], axis=1))  # [p,2,f,T]

        dpad = np.zeros((T, n_pad), np.float32)
        dpad[:, :nc_n] = dobs[:, sl]
        dc = np.ascontiguousarray(dpad.reshape(T, p, f_core).transpose(1, 2, 0))

        dpc = np.zeros((n_pad,), np.float32)
        dpc[:nc_n] = dpred[sl]
        in_maps.append({"wp": wp, "dobs": dc, "dpred": dpc})
    return in_maps


def make_consts(beta, p=P):
    steps = _steps_np()
    row = -(np.float32(beta) * steps).astype(np.float32)   # [BINS]
    return np.ascontiguousarray(np.broadcast_to(row, (p, BINS)))


def kernel(inptasksobs=None, inptasksperf=None, inptaskspred=None,
           num_obs_tasks=None, tasksobsids=None, taskspredids=None,
           difficulties_obs=None, difficulties_pred=None,
           betas=None, zetas=None, **_):
    beta = float(np.float32(np.asarray(betas).reshape(-1)[0]))
    zeta = np.float32(np.asarray(zetas).reshape(-1)[0])
    mq = float(np.float32(-(zeta * zeta)))

    nc = _get_nc(beta, mq)
    in_maps = make_in_maps(inptasksperf, difficulties_obs, difficulties_pred)
    consts = make_consts(beta)
    for m in in_maps:
        m["consts"] = consts
    res = bass_utils.run_bass_kernel_spmd(nc, in_maps,
                                          core_ids=list(range(NCORES)))
    nc_n = N_TOTAL // NCORES
    parts = [np.asarray(r["out"]).reshape(-1)[:nc_n] for r in res.results]
    return np.concatenate(parts).reshape(N_TOTAL, 1).astype(np.float32)


if __name__ == "__main__":
    rng = np.random.default_rng(0)
    ins = {
        "inptasksperf": rng.integers(0, 2, (T, N_TOTAL, 2)).astype(np.int32),
        "difficulties_obs": (0.9 * rng.random((T, N_TOTAL, 1))).astype(np.float32),
        "difficulties_pred": (0.9 * rng.random((N_TOTAL, 1))).astype(np.float32),
        "betas": np.array([7.0], np.float32),
        "zetas": np.array([0.5], np.float32),
    }
    out = kernel(**ins)
    print(out.shape, out.dtype, out[:5, 0])


# revision 22
# speedup vs baseline: 1.6029x; 1.0000x over previous
"""Trainium2 Bass kernel for nn_BidirectionalTrustModel (histogram_binning).

Computes, per observation sequence n (N = 500000, T = 20, BINS = 12):
  1. capability edge c[n]: sequential fold over t of
       c = max(c, d)  if perf==[0,1]
       c = min(c, d)  if perf[...,0]==1
       c              otherwise
  2. trust[n] = sum_k t_k * m_k / sum_k m_k  over 12 bin centers s_k,
       m_k = (c <= s_k),  t_k = (1 + exp(beta*(dpred - s_k)))**(-zeta^2)

Only inptasksperf, difficulties_obs, difficulties_pred, betas, zetas are used
(the other inputs are dead in the reference computation).

Device mapping (pure data parallel over 8 cores, no collectives):
  - per-core slice of 62500 sequences, padded to 62720 = 128 partitions x 490
  - host relayout to t-inner [128, 490, 20] planes; perf recoded losslessly
    to int8 planes w = p1 - p0 - 1 and p0 (cuts perf HBM 8B -> 2B per cell)
  - phase A (exact arithmetic: the fold's trigger value is exactly d because
    only +0.0 / *1.0 ever touch d):
      lo = d + w              in {d (max-step), d-1, d-2}
      hi = (p0 < 1) + d       in {d (min-step), d+1}
      slot-0 override lo0 = hi0 = d*m0 forces state = step0(0) regardless of
      scan carry-in (min(max(v, s), v) == v), so sequences pack back-to-back
      with NO reset slots and ONE contiguous tensor_tensor_scan(max, min)
      over [128, nseq*20] computes every capability edge.
  - phase B: per bin t_k via exp/ln/exp ACT chain (one act table), exact
    is_le masks (bf16), fused (C<=s_k)*t_k scalar_tensor_tensor, contiguous
    slab-add reductions, vector.reciprocal.
"""
import sys

if "/opt/trn_rl_repo" not in sys.path:
    sys.path.insert(0, "/opt/trn_rl_repo")

from contextlib import ExitStack

import numpy as np

import concourse.bacc as bacc
import concourse.bass as bass
import concourse.mybir as mybir
import concourse.tile as tile
from concourse import bass_utils
from concourse.hw_specs import get_activation_tables as _orig_act_tables


def _combined_act_tables(arch):
    """Keep only natural_log_exp_and_others usable (positions preserved —
    the list index is the act_func_set_id) so Exp/Ln/Copy all resolve to ONE
    table: no ACT_TABLE_LOAD thrash between exp and ln."""
    t = _orig_act_tables(arch)
    return {k: (v if k == "natural_log_exp_and_others" else set())
            for k, v in t.items()}


bacc.get_activation_tables = _combined_act_tables

N_TOTAL = 500000
T = 20
BINS = 12
NCORES = 8
P = 128

AOT = mybir.AluOpType
ACTF = mybir.ActivationFunctionType
F32 = mybir.dt.float32
BF16 = mybir.dt.bfloat16
I8 = mybir.dt.int8


def _steps_np():
    # bit-exact match of jnp: (arange(BINS) + 0.5) / BINS in f32
    return (np.arange(BINS, dtype=np.float32) + np.float32(0.5)) / np.float32(BINS)


def build_nc(beta: float, mq: float, n_pad: int, ft, ncores: int = NCORES,
             p: int = P):
    """Build the Bass module. n_pad = per-core padded sequence count
    (= p * f_core), ft = tile widths (int or list, sequences per partition
    per tile; first tiles smaller to prime the DMA/compute pipeline)."""
    f_core = n_pad // p
    assert f_core * p == n_pad
    if isinstance(ft, int):
        assert f_core % ft == 0
        fts = [ft] * (f_core // ft)
    else:
        fts = list(ft)
        assert sum(fts) == f_core
    steps = _steps_np()

    nc = bacc.Bacc("TRN2", target_bir_lowering=False, debug=False,
                   enable_asserts=False, num_devices=ncores)

    d_wp = nc.dram_tensor("wp", [p, 2, f_core, T], I8, kind="ExternalInput").ap()
    d_m0 = nc.dram_tensor("m0", [p, f_core], I8, kind="ExternalInput").ap()
    d_d = nc.dram_tensor("dobs", [p, f_core, T], F32, kind="ExternalInput").ap()
    d_dpred = nc.dram_tensor("dpred", [n_pad], F32, kind="ExternalInput").ap()
    d_consts = nc.dram_tensor("consts", [p, BINS], F32,
                              kind="ExternalInput").ap()
    d_out = nc.dram_tensor("out", [p, f_core], F32, kind="ExternalOutput").ap()

    with tile.TileContext(nc) as tc:
        with ExitStack() as ctx:
            inpool = ctx.enter_context(tc.tile_pool(name="in", bufs=len(fts)))
            work = ctx.enter_context(tc.tile_pool(name="work", bufs=2))
            keep = ctx.enter_context(tc.tile_pool(name="keep", bufs=1))

            C = keep.tile([p, f_core], F32, tag="C")
            DP = keep.tile([p, f_core], F32, tag="DP")
            nc.scalar.dma_start(DP[:], d_dpred.rearrange("(p n) -> p n", p=p))
            CB = keep.tile([p, BINS], F32, tag="CB")
            nc.scalar.dma_start(CB[:], d_consts)
            M0 = keep.tile([p, f_core], I8, tag="M0")
            nc.scalar.dma_start(M0[:], d_m0)

            base = 0
            for j, ftj in enumerate(fts):
                sl = slice(base, base + ftj)
                base += ftj
                FT20 = ftj * T
                Dt = inpool.tile([p, FT20], F32, tag="Dt")
                nc.sync.dma_start(
                    Dt[:].rearrange("p (n t) -> p n t", t=T), d_d[:, sl, :])
                WPt = inpool.tile([p, 2 * FT20], I8, tag="WPt")
                nc.scalar.dma_start(
                    WPt[:].rearrange("p (c n t) -> p c n t", c=2, t=T),
                    d_wp[:, :, sl, :])
                Wt = WPt[:, 0:FT20]
                Qt = WPt[:, FT20:2 * FT20]

                LO = work.tile([p, FT20], F32, tag="LO")
                HI = work.tile([p, FT20], F32, tag="HI")
                # lo = d + w ; hi = d + q, q = 1-p0  (flat, fully contiguous)
                nc.vector.tensor_tensor(LO[:], Dt[:], Wt, AOT.add)
                nc.vector.tensor_tensor(HI[:], Dt[:], Qt, AOT.add)
                # slot-0 self-reset override: lo0 = hi0 = d0 * m0
                d0 = Dt[:].rearrange("p (n t) -> p n t", t=T)[:, :, 0]
                lo0 = LO[:].rearrange("p (n t) -> p n t", t=T)[:, :, 0]
                hi0 = HI[:].rearrange("p (n t) -> p n t", t=T)[:, :, 0]
                nc.vector.tensor_tensor(lo0, M0[:, sl], d0, AOT.mult)
                nc.scalar.copy(hi0, lo0)

                CS = work.tile([p, FT20], F32, tag="CS")
                nc.vector.tensor_tensor_scan(CS[:], LO[:], HI[:], 0.0,
                                             AOT.max, AOT.min)
                # c for this tile: scan state after step T-1
                cview = CS[:].rearrange("p (n t) -> p n t", t=T)[:, :, T - 1]
                nc.scalar.copy(C[:, sl], cview)

            # ---- phase B ----
            SP = keep.tile([p, BINS * f_core], F32, tag="SP")
            SPv = SP[:].rearrange("p (k n) -> p k n", k=BINS)
            for k in range(BINS):
                nc.scalar.activation(SPv[:, k, :], DP[:], ACTF.Exp,
                                     bias=CB[:, k:k + 1],
                                     scale=float(np.float32(beta)))
            nc.scalar.activation(SP[:], SP[:], ACTF.Ln, bias=1.0)
            nc.scalar.activation(SP[:], SP[:], ACTF.Exp,
                                 scale=float(np.float32(mq)))

            M = keep.tile([p, BINS * f_core], BF16, tag="M")
            Mv = M[:].rearrange("p (k n) -> p k n", k=BINS)
            for k in range(BINS):
                nc.vector.tensor_scalar(Mv[:, k, :], C[:], float(steps[k]),
                                        None, AOT.is_le)

            def pair_tree(src_v, nbins, dtype, tag):
                """Pairwise-tree sum over the k axis of a [p, k, n] view."""
                v = src_v
                nk = nbins
                lvl = 0
                while nk > 1:
                    half = nk // 2
                    out_t = keep.tile([p, (half + nk % 2) * f_core], dtype,
                                      tag=f"{tag}L{lvl}")
                    ov = out_t[:].rearrange("p (k n) -> p k n", k=half + nk % 2)
                    nc.vector.tensor_tensor(ov[:, 0:half, :],
                                            v[:, 0:2 * half:2, :],
                                            v[:, 1:2 * half:2, :], AOT.add)
                    if nk % 2:
                        nc.scalar.copy(ov[:, half, :], v[:, nk - 1, :])
                    v, nk, lvl = ov, half + nk % 2, lvl + 1
                return v[:, 0, :]

            # msum path first so ACT's reciprocal overlaps the tsum work
            msum = pair_tree(Mv, BINS, BF16, "ms")
            # 1/msum on ACT: exp(-ln(msum)) — exact ints 1..12, ~ulp error
            rec = keep.tile([p, f_core], F32, tag="rec")
            nc.scalar.activation(rec[:], msum, ACTF.Ln)
            nc.scalar.activation(rec[:], rec[:], ACTF.Exp, scale=-1.0)

            # masked t_k in place, one big op: SP = SP * M
            nc.vector.tensor_tensor(SP[:], SP[:], M[:], AOT.mult)
            tsum = pair_tree(SPv, BINS, F32, "ts")
            OUT = keep.tile([p, f_core], F32, tag="OUT")
            nc.vector.tensor_tensor(OUT[:], tsum, rec[:], AOT.mult)
            nc.sync.dma_start(d_out, OUT[:])

    nc.compile()
    return nc


_CACHE: dict = {}


def _get_nc(beta: float, mq: float):
    key = (beta, mq)
    if key not in _CACHE:
        _CACHE[key] = build_nc(beta, mq, n_pad=62720,
                               ft=[28, 70, 98, 98, 98, 98])
    return _CACHE[key]


def make_in_maps(inptasksperf, difficulties_obs, difficulties_pred,
                 n_total=N_TOTAL, ncores=NCORES, n_pad=62720, p=P):
    """Host-side shard + pad + t-inner relayout + int8 recoding."""
    perf = np.asarray(inptasksperf)
    dobs = np.asarray(difficulties_obs, dtype=np.float32)[..., 0]    # [T, N]
    dpred = np.asarray(difficulties_pred, dtype=np.float32)[..., 0]  # [N]
    f_core = n_pad // p
    nc_n = n_total // ncores

    w_all = (perf[..., 1] - perf[..., 0] - 1).astype(np.int8)        # [T, N]
    p0_all = perf[..., 0].astype(np.int8)                            # [T, N]

    in_maps = []
    for c in range(ncores):
        sl = slice(c * nc_n, (c + 1) * nc_n)

        wpad = np.full((T, n_pad), -1, np.int8)
        wpad[:, :nc_n] = w_all[:, sl]
        wc = wpad.reshape(T, p, f_core).transpose(1, 2, 0)

        qpad = np.ones((T, n_pad), np.int8)
        qpad[:, :nc_n] = 1 - p0_all[:, sl]
        qc = qpad.reshape(T, p, f_core).transpose(1, 2, 0)

        wp = np.ascontiguousarray(np.stack([wc, qc], axis=1))   # [p,2,f,T]
        m0c = np.ascontiguousarray(
            (wpad[0].reshape(p, f_core) == 0).astype(np.int8))

        dpad = np.zeros((T, n_pad), np.float32)
        dpad[:, :nc_n] = dobs[:, sl]
        dc = np.ascontiguousarray(dpad.reshape(T, p, f_core).transpose(1, 2, 0))

        dpc = np.zeros((n_pad,), np.float32)
        dpc[:nc_n] = dpred[sl]
        in_maps.append({"wp": wp, "m0": m0c, "dobs": dc, "dpred": dpc})
    return in_maps


def make_consts(beta, p=P):
    steps = _steps_np()
    row = -(np.float32(beta) * steps).astype(np.float32)   # [BINS]
    return np.ascontiguousarray(np.broadcast_to(row, (p, BINS)))


def kernel(inptasksobs=None, inptasksperf=None, inptaskspred=None,
           num_obs_tasks=None, tasksobsids=None, taskspredids=None,
           difficulties_obs=None, difficulties_pred=None,
           betas=None, zetas=None, **_):
    beta = float(np.float32(np.asarray(betas).reshape(-1)[0]))
    zeta = np.float32(np.asarray(zetas).reshape(-1)[0])
    mq = float(np.float32(-(zeta * zeta)))

    nc = _get_nc(beta, mq)
    in_maps = make_in_maps(inptasksperf, difficulties_obs, difficulties_pred)
    consts = make_consts(beta)
    for m in in_maps:
        m["consts"] = consts
    res = bass_utils.run_bass_kernel_spmd(nc, in_maps,
                                          core_ids=list(range(NCORES)))
    nc_n = N_TOTAL // NCORES
    parts = [np.asarray(r["out"]).reshape(-1)[:nc_n] for r in res.results]
    return np.concatenate(parts).reshape(N_TOTAL, 1).astype(np.float32)


if __name__ == "__main__":
    rng = np.random.default_rng(0)
    ins = {
        "inptasksperf": rng.integers(0, 2, (T, N_TOTAL, 2)).astype(np.int32),
        "difficulties_obs": (0.9 * rng.random((T, N_TOTAL, 1))).astype(np.float32),
        "difficulties_pred": (0.9 * rng.random((N_TOTAL, 1))).astype(np.float32),
        "betas": np.array([7.0], np.float32),
        "zetas": np.array([0.5], np.float32),
    }
    out = kernel(**ins)
    print(out.shape, out.dtype, out[:5, 0])
